# revision 10
# baseline (speedup 1.0000x reference)
"""Sparse (sigmoid) attention block on 8 TRN2 NeuronCores.

Sharding: core c = (batch b=c//2, head-half hh=c%2).  Each core computes
QKV projection + RoPE + causal sigmoid-attention for its 6 heads over the
full 2048-row sequence of its batch, then LayerNorm-gates ONLY its own
384-dim hidden half:  LN statistics are exchanged with the pair partner
via a tiny (4 KB) AllReduce, the output projection is split along the
contraction dim, and a ReduceScatter(add) of the partial products gives
each core the final rows of its half.  This removes the baseline's big
attention-output AllGather and halves the duplicated U-projection /
LayerNorm work.

Layouts: Q^T/K^T are produced DIRECTLY in transposed [head-pair-dim, seq]
layout by making the projection weights PE-stationary; RoPE's rotate-half
becomes a host-built 128x128 permutation matmul (P @ Q^T) so no PE
transposes are needed anywhere.  Causal structure is exploited at 128-row
granularity inside each 512-query block: diagonal key-chunks trim their
scores / sigmoid / A@V work to the unmasked column range.  All heavy
compute in bf16 with f32 PSUM accumulation.
"""

import numpy as np
import ml_dtypes

import concourse.bass as bass
import concourse.bacc as bacc
import concourse.mybir as mybir
import concourse.tile as tile
from concourse import bass_utils

BF16 = mybir.dt.bfloat16
F32 = mybir.dt.float32
AF = mybir.ActivationFunctionType

S = 2048          # sequence length
HID = 768         # hidden
D = 64            # head dim
NH = 6            # heads per core
NPAIR = 3         # head pairs per core
NRB = 4           # row blocks of 512
RB = 512
LN_EPS = 1e-8
N_CORES = 8


def _rope_tables():
    inv_freq = 1.0 / (10000.0 ** (np.arange(0, D, 2, dtype=np.float64) / D))
    t = np.arange(S, dtype=np.float64)
    freqs = np.outer(t, inv_freq)                      # [S, 32]
    emb = np.concatenate([freqs, freqs], axis=-1)      # [S, 64]
    return np.cos(emb).astype(np.float32), np.sin(emb).astype(np.float32)


def build_nc(ndev, pairs):
    """Emit the per-core Bass/Tile graph (identical for every core)."""
    nc = bacc.Bacc("TRN2", target_bir_lowering=False, debug=False,
                   num_devices=ndev)

    def din(name, shape, dt):
        return nc.dram_tensor(name, shape, dt, kind="ExternalInput").ap()

    xT = din("xT", [HID, S], BF16)
    wq = din("wq", [HID, 384], BF16)
    wk = din("wk", [HID, 384], BF16)
    wv = din("wv", [HID, 384], BF16)
    wu = din("wu", [HID, 384], BF16)                   # own-half U cols
    wo = din("wo", [384, HID], BF16)                   # own rows, g-folded
    cosT = din("cosT", [128, S], BF16)
    sinT = din("sinT", [128, S], BF16)                 # sign-folded sin^T
    pmat = din("pmat", [128, 128], BF16)               # rotate-half perm
    maskb = din("maskb", [128, 128], BF16)             # j>=i upper-tri
    ones_k = din("ones_k", [128, 1], BF16)
    residT = din("residT", [384, S], F32)              # x^T half + b_out
    out = nc.dram_tensor("out", [384, S], F32, kind="ExternalOutput").ap()

    xT_r = xT.rearrange("(k p) s -> p k s", p=128)     # [128, 6, S]
    wq_r = wq.rearrange("(k p) c -> p k c", p=128)     # [128, 6, 384]
    wk_r = wk.rearrange("(k p) c -> p k c", p=128)
    wv_r = wv.rearrange("(k p) c -> p k c", p=128)
    wu_r = wu.rearrange("(k p) c -> p k c", p=128)
    wo_r = wo.rearrange("(k p) c -> p k c", p=128)     # [128, 3, 768]
    residT_r = residT.rearrange("(c p) s -> p c s", p=128)  # [128, 3, S]
    out_r = out.rearrange("(c p) s -> p c s", p=128)

    with tile.TileContext(nc) as tc:
        _emit(nc, tc, pairs, xT_r, wq_r, wk_r, wv_r, wu_r, wo_r,
              cosT, sinT, pmat, maskb, ones_k, residT_r, out_r)
    nc.compile()
    return nc


def _emit(nc, tc, pairs, xT_r, wq_r, wk_r, wv_r, wu_r, wo_r,
          cosT, sinT, pmat, maskb, ones_k, residT_r, out_r):
    from contextlib import ExitStack
    es = ExitStack()
    with es:
        # ---- resident SBUF tensors -----------------------------------
        res = es.enter_context(tc.tile_pool(name="resident", bufs=1))
        xT_sb = res.tile([128, 6, S], BF16, tag="xT")
        wq_sb = res.tile([128, 6, 384], BF16, tag="wq")
        wk_sb = res.tile([128, 6, 384], BF16, tag="wk")
        wv_sb = res.tile([128, 6, 384], BF16, tag="wv")
        wu_sb = res.tile([128, 6, 384], BF16, tag="wu")
        wo_sb = res.tile([128, 3, HID], BF16, tag="wo")
        cosT_sb = res.tile([128, S], BF16, tag="cosT")
        sinT_sb = res.tile([128, S], BF16, tag="sinT")
        pmat_sb = res.tile([128, 128], BF16, tag="pmat")
        maskb_sb = res.tile([128, 128], BF16, tag="maskb")
        ones_k_sb = res.tile([128, 1], BF16, tag="onesk")
        warm_sb = res.tile([128, 128], BF16, tag="warm")
        qt_sb = [res.tile([128, NPAIR, RB], BF16, tag=f"qt{i}", name=f"qt{i}")
                 for i in range(NRB)]                  # Q^T slabs (roped)
        kt_sb = [res.tile([128, NPAIR, RB], BF16, tag=f"kt{i}", name=f"kt{i}")
                 for i in range(NRB)]                  # K^T slabs (roped)
        v_sb = [res.tile([128, 4, NH * D], BF16, tag=f"v{i}", name=f"v{i}")
                for i in range(NRB)]                   # V row-major slabs
        ut_sb = res.tile([128, NPAIR, S], BF16, tag="ut")   # silu(U)^T half
        ao_sb = [res.tile([128, NPAIR, RB], BF16, tag=f"ao{i}", name=f"ao{i}")
                 for i in range(NRB)]                  # attn out^T half

        # critical-path loads first: wq + first x block start the PE
        nc.gpsimd.memset(warm_sb[:], 0.0)
        for k in range(6):
            nc.sync.dma_start(out=wq_sb[:, k, :], in_=wq_r[:, k, :])
        for k in range(6):
            nc.sync.dma_start(out=xT_sb[:, k, 0:RB], in_=xT_r[:, k, 0:RB])
        nc.sync.dma_start(out=pmat_sb[:], in_=pmat[:])
        nc.sync.dma_start(out=cosT_sb[:], in_=cosT[:])
        nc.sync.dma_start(out=sinT_sb[:], in_=sinT[:])
        for k in range(6):
            nc.sync.dma_start(out=wk_sb[:, k, :], in_=wk_r[:, k, :])
            nc.sync.dma_start(out=wv_sb[:, k, :], in_=wv_r[:, k, :])
            nc.sync.dma_start(out=wu_sb[:, k, :], in_=wu_r[:, k, :])
        nc.sync.dma_start(out=maskb_sb[:], in_=maskb[:])
        nc.sync.dma_start(out=ones_k_sb[:], in_=ones_k[:])
        for rb in range(1, NRB):
            for k in range(6):
                nc.sync.dma_start(out=xT_sb[:, k, rb * RB:(rb + 1) * RB],
                                  in_=xT_r[:, k, rb * RB:(rb + 1) * RB])
        for k in range(3):
            nc.sync.dma_start(out=wo_sb[:, k, :], in_=wo_r[:, k, :])

        # ---- pools ---------------------------------------------------
        dram = es.enter_context(tc.tile_pool(name="ccdram", bufs=4,
                                             space="DRAM"))
        projp = es.enter_context(tc.tile_pool(name="projp", bufs=2,
                                              space="PSUM"))      # 2 banks
        scp = es.enter_context(tc.tile_pool(name="scp", bufs=2,
                                            space="PSUM"))        # 4 banks
        avp = es.enter_context(tc.tile_pool(name="avp", bufs=1,
                                            space="PSUM"))        # 1 bank
        opo = es.enter_context(tc.tile_pool(name="opo", bufs=1,
                                            space="PSUM"))        # 1 bank
        psb = es.enter_context(tc.tile_pool(name="psb", bufs=2))
        atp = es.enter_context(tc.tile_pool(name="atp", bufs=10))
        esb = es.enter_context(tc.tile_pool(name="esb", bufs=2))
        ssb = es.enter_context(tc.tile_pool(name="ssb", bufs=1))

        eps_t = ssb.tile([1, 1], F32, tag="eps")
        nc.gpsimd.memset(eps_t[:], LN_EPS)

        # PE warm-up: ramp the clock while the first DMAs land
        wp = projp.tile([128, RB], F32, tag="pq", name="warm")
        for i in range(16):
            nc.tensor.matmul(wp[:, 0:128], warm_sb[:], warm_sb[:],
                             start=True, stop=True)

        # ---------------- per-phase emitters --------------------------
        def proj_qk(rb):
            """Q^T,K^T directly via W-stationary matmuls + perm-RoPE."""
            r0, r1 = rb * RB, (rb + 1) * RB
            units = [(wq_sb, qt_sb[rb], p) for p in range(NPAIR)]
            units += [(wk_sb, kt_sb[rb], p) for p in range(NPAIR)]
            # interleave: Q(p)/K(p) pq accumulation covers the previous
            # unit's PSUM->SBUF copy so the perm matmul never stalls
            staged = []
            for w_sb, dst, p in units:
                pq = projp.tile([128, RB], F32, tag="pq", name=f"pq{p}")
                for k in range(6):
                    nc.tensor.matmul(pq[:], w_sb[:, k, p * 128:(p + 1) * 128],
                                     xT_sb[:, k, r0:r1],
                                     start=(k == 0), stop=(k == 5))
                qsb = psb.tile([128, RB], BF16, tag="qsb")
                nc.scalar.copy(qsb[:], pq[:])
                staged.append((qsb, dst, p))
                if len(staged) == 2:
                    _finish_qk(staged.pop(0), r0, r1)
            while staged:
                _finish_qk(staged.pop(0), r0, r1)

        def _finish_qk(st, r0, r1):
            qsb, dst, p = st
            pperm = projp.tile([128, RB], F32, tag="pq", name="pperm")
            nc.tensor.matmul(pperm[:], pmat_sb[:], qsb[:],
                             start=True, stop=True)
            t1 = psb.tile([128, RB], BF16, tag="t1")
            t2 = psb.tile([128, RB], BF16, tag="t2")
            nc.vector.tensor_mul(t1[:], qsb[:], cosT_sb[:, r0:r1])
            nc.vector.tensor_mul(t2[:], pperm[:], sinT_sb[:, r0:r1])
            nc.vector.tensor_add(dst[:, p, :], t1[:], t2[:])

        def proj_vu(rb):
            r0 = rb * RB
            for rt4 in range(4):
                c0 = r0 + rt4 * 128
                pv = projp.tile([128, RB], F32, tag="pq", name="pv")
                for k in range(6):
                    nc.tensor.matmul(pv[:, 0:384], xT_sb[:, k, c0:c0 + 128],
                                     wv_sb[:, k, :],
                                     start=(k == 0), stop=(k == 5))
                nc.scalar.copy(v_sb[rb][:, rt4, :], pv[:, 0:384])
            for ct in range(NPAIR):
                pu = projp.tile([128, RB], F32, tag="pq", name="pu")
                for k in range(6):
                    nc.tensor.matmul(pu[:], wu_sb[:, k, ct * 128:(ct + 1) * 128],
                                     xT_sb[:, k, r0:r0 + RB],
                                     start=(k == 0), stop=(k == 5))
                usig = psb.tile([128, RB], BF16, tag="usig")
                nc.scalar.activation(usig[:], pu[:], AF.Sigmoid)
                nc.vector.tensor_mul(ut_sb[:, ct, r0:r0 + RB], usig[:], pu[:])

        def attn(qb):
            """Causal sigmoid attention for query block qb, all pairs.

            Emits the per-pair LN-stat matmuls right after each pair's
            attention output so only the last pair's stats trail the
            attention end."""
            nkc = 4 * qb + 4
            q0 = qb * RB
            ssum = projp.tile([1, RB], F32, tag="pq", name="ssum")
            qsum = projp.tile([1, RB], F32, tag="pq", name="qsum")
            for p in range(NPAIR):
                av = avp.tile([128, RB], F32, tag="av")
                ats = {}

                def _av(kc):
                    t = kc - 4 * qb
                    w0 = max(t, 0) * 128
                    at = ats.pop(kc)
                    for h01 in range(2):
                        b0 = 64 * h01
                        nc.tensor.matmul(
                            av[b0:b0 + 64, w0:RB],
                            v_sb[kc // 4][:, kc % 4,
                                          (2 * p + h01) * D:(2 * p + h01 + 1) * D],
                            at[:, h01, w0:RB],
                            start=(kc == 0), stop=(kc == nkc - 1),
                            skip_group_check=True)

                for kc in range(nkc):
                    t = kc - 4 * qb          # >=0: diagonal-region chunk
                    w0 = max(t, 0) * 128
                    sc = scp.tile([128, 2, RB], F32, tag="sc")
                    at = atp.tile([128, 2, RB], BF16, tag="at")
                    kslc = kt_sb[kc // 4]
                    c0 = (kc % 4) * 128
                    for h01 in range(2):
                        b0 = 64 * h01
                        nc.tensor.matmul(
                            sc[:, h01, w0:RB],
                            kslc[b0:b0 + 64, p, c0:c0 + 128],
                            qt_sb[qb][b0:b0 + 64, p, w0:RB],
                            start=True, stop=True)
                    nc.scalar.activation(at[:, :, w0:RB], sc[:, :, w0:RB],
                                         AF.Sigmoid, scale=0.125)
                    if t >= 0:
                        for h01 in range(2):
                            nc.vector.tensor_mul(at[:, h01, w0:w0 + 128],
                                                 at[:, h01, w0:w0 + 128],
                                                 maskb_sb[:])
                    ats[kc] = at
                    if kc >= 8:              # bound live `at` tiles
                        _av(kc - 8)
                for kc in sorted(ats):
                    _av(kc)
                nc.vector.tensor_copy(ao_sb[qb][:, p, :], av[:])
                # per-pair LN stat contributions (sum, sum of squares)
                sq = psb.tile([128, RB], BF16, tag="sq")
                nc.vector.tensor_mul(sq[:], ao_sb[qb][:, p, :],
                                     ao_sb[qb][:, p, :])
                nc.tensor.matmul(ssum[:], ones_k_sb[:], ao_sb[qb][:, p, :],
                                 start=(p == 0), stop=(p == NPAIR - 1))
                nc.tensor.matmul(qsum[:], ones_k_sb[:], sq[:],
                                 start=(p == 0), stop=(p == NPAIR - 1))
            # ship partial stats to the pair partner (4 KB AllReduce)
            stats_sb = ssb.tile([1, 2 * RB], F32, tag="statsb", name=f"stb{qb}")
            nc.scalar.copy(stats_sb[:, 0:RB], ssum[:])
            nc.scalar.copy(stats_sb[:, RB:2 * RB], qsum[:])
            ar_in = dram.tile([1, 2 * RB], F32, tag="arin")
            ar_out = dram.tile([1, 2 * RB], F32, tag="arout")
            nc.gpsimd.dma_start(out=ar_in[:], in_=stats_sb[:])
            nc.gpsimd.collective_compute(
                "AllReduce", mybir.AluOpType.add, replica_groups=pairs,
                ins=[ar_in.opt()], outs=[ar_out.opt()])
            st2 = ssb.tile([1, 2 * RB], F32, tag="st2", name=f"st2{qb}")
            nc.sync.dma_start(out=st2[:], in_=ar_out[:])
            return st2

        def epilogue(rb, st2):
            """LN + gate own half, out-proj partials, ReduceScatter, store."""
            r0, r1 = rb * RB, (rb + 1) * RB
            mu = ssb.tile([1, RB], F32, tag="mu", name=f"mu{rb}")
            musq = ssb.tile([1, RB], F32, tag="musq", name=f"musq{rb}")
            var = ssb.tile([1, RB], F32, tag="var", name=f"var{rb}")
            std = ssb.tile([1, RB], F32, tag="musq", name=f"std{rb}")
            rstd = ssb.tile([1, RB], F32, tag="var", name=f"rstd{rb}")
            nc.vector.tensor_scalar_mul(mu[:], st2[:, 0:RB], 1.0 / HID)
            nc.vector.tensor_mul(musq[:], mu[:], mu[:])
            nc.vector.scalar_tensor_tensor(
                var[:], st2[:, RB:2 * RB], 1.0 / HID, musq[:],
                op0=mybir.AluOpType.mult, op1=mybir.AluOpType.subtract)
            nc.scalar.activation(std[:], var[:], AF.Sqrt, bias=eps_t[:])
            nc.vector.reciprocal_approx_fast(rstd[:], std[:])
            mu_s = esb.tile([128, RB], F32, tag="mus", bufs=1)
            rs_s = esb.tile([128, RB], F32, tag="rss", bufs=1)
            nc.gpsimd.partition_broadcast(mu_s[:], mu[:])
            nc.gpsimd.partition_broadcast(rs_s[:], rstd[:])
            gated = esb.tile([128, NPAIR, RB], BF16, tag="gated", bufs=1)
            for ct in range(NPAIR):
                d1 = esb.tile([128, RB], BF16, tag="d1")
                d2 = esb.tile([128, RB], BF16, tag="d2")
                nc.vector.tensor_sub(d1[:], ao_sb[rb][:, ct, :], mu_s[:])
                nc.vector.tensor_mul(d2[:], d1[:], rs_s[:])
                nc.vector.tensor_mul(gated[:, ct, :], d2[:],
                                     ut_sb[:, ct, r0:r1])
            pob = esb.tile([128, 6, RB], BF16, tag="pob", bufs=1)
            for oc in range(6):
                po = opo.tile([128, RB], F32, tag="po")
                for ct in range(NPAIR):
                    nc.tensor.matmul(po[:], wo_sb[:, ct, oc * 128:(oc + 1) * 128],
                                     gated[:, ct, :],
                                     start=(ct == 0), stop=(ct == NPAIR - 1))
                nc.vector.tensor_copy(pob[:, oc, :], po[:])
            rs_in = dram.tile([6, 128, RB], BF16, tag="rsin")
            rs_out = dram.tile([3, 128, RB], BF16, tag="rsout")
            nc.gpsimd.dma_start(out=rs_in.rearrange("o p j -> p o j"),
                                in_=pob[:])
            nc.gpsimd.collective_compute(
                "ReduceScatter", mybir.AluOpType.add, replica_groups=pairs,
                ins=[rs_in.opt()], outs=[rs_out.opt()])
            rsb = esb.tile([128, 3, RB], BF16, tag="rsb")
            nc.sync.dma_start(out=rsb[:],
                              in_=rs_out.rearrange("o p j -> p o j"))
            rt_t = esb.tile([128, 3, RB], F32, tag="resid", bufs=1)
            nc.sync.dma_start(out=rt_t[:], in_=residT_r[:, :, r0:r1])
            o_t = esb.tile([128, 3, RB], F32, tag="osb", bufs=1)
            nc.vector.tensor_add(o_t[:], rsb[:], rt_t[:])
            nc.gpsimd.dma_start(out=out_r[:, :, r0:r1], in_=o_t[:])

        # ---------------- schedule ------------------------------------
        # epilogue(rb) is emitted one block behind attention so the
        # stats AllReduce latency hides under the next attention block.
        st = {}
        proj_qk(0)
        proj_vu(0)
        st[0] = attn(0)
        proj_qk(1)
        proj_vu(1)
        st[1] = attn(1)
        epilogue(0, st[0])
        proj_qk(2)
        proj_vu(2)
        st[2] = attn(2)
        epilogue(1, st[1])
        proj_qk(3)
        proj_vu(3)
        st[3] = attn(3)
        epilogue(2, st[2])
        epilogue(3, st[3])


# ---------------------------------------------------------------------------
# host side
# ---------------------------------------------------------------------------

def prep_inputs(x, attn_mask, W_proj, b_proj, ln_gamma, ln_beta, W_out, b_out):
    x = np.asarray(x, dtype=np.float32)
    W_proj = np.asarray(W_proj, dtype=np.float32)
    b_proj = np.asarray(b_proj, dtype=np.float32)
    ln_gamma = np.asarray(ln_gamma, dtype=np.float32)
    ln_beta = np.asarray(ln_beta, dtype=np.float32)
    W_out = np.asarray(W_out, dtype=np.float32)
    b_out = np.asarray(b_out, dtype=np.float32)

    tril = np.tril(np.ones((S, S), dtype=bool))
    am = np.asarray(attn_mask)
    if not all(np.array_equal(am[b], tril) for b in range(am.shape[0])):
        raise ValueError("kernel specialized for causal attn_mask")
    if np.any(b_proj != 0) or np.any(ln_beta != 0):
        raise ValueError("kernel specialized for zero b_proj / ln_beta")

    bf = ml_dtypes.bfloat16
    cos, sin = _rope_tables()
    sinN = sin.copy()
    sinN[:, 0:32] = -sinN[:, 0:32]
    cosT = np.tile(cos.T, (2, 1)).astype(bf)           # [128, S]
    sinT = np.tile(sinN.T, (2, 1)).astype(bf)

    pmat = np.zeros((128, 128), dtype=np.float32)      # rotate-half perm
    for h in range(2):
        b0 = 64 * h
        for d in range(64):
            pmat[b0 + d, b0 + (d + 32) % 64] = 1.0
    pmat = pmat.astype(bf)

    maskb = np.triu(np.ones((128, 128), dtype=np.float32)).astype(bf)
    ones_k = np.ones((128, 1), dtype=bf)

    Wg = (ln_gamma[:, None] * W_out).astype(np.float32)   # gamma folded
    U_c, V_c, Q_c, K_c = 0, HID, 2 * HID, 3 * HID

    in_maps = []
    for c in range(N_CORES):
        b, hh = c // 2, c % 2
        h0 = NH * hh * D                               # 384*hh col offset
        xTb = x[b].T                                   # [768, 2048]
        residT = (xTb[hh * 384:(hh + 1) * 384, :]
                  + b_out[hh * 384:(hh + 1) * 384, None]).astype(np.float32)
        in_maps.append(dict(
            xT=np.ascontiguousarray(xTb).astype(bf),
            wq=np.ascontiguousarray(W_proj[:, Q_c + h0:Q_c + h0 + 384]).astype(bf),
            wk=np.ascontiguousarray(W_proj[:, K_c + h0:K_c + h0 + 384]).astype(bf),
            wv=np.ascontiguousarray(W_proj[:, V_c + h0:V_c + h0 + 384]).astype(bf),
            wu=np.ascontiguousarray(W_proj[:, U_c + h0:U_c + h0 + 384]).astype(bf),
            wo=np.ascontiguousarray(Wg[hh * 384:(hh + 1) * 384, :]).astype(bf),
            cosT=cosT, sinT=sinT, pmat=pmat, maskb=maskb, ones_k=ones_k,
            residT=np.ascontiguousarray(residT),
        ))
    return in_maps


def assemble(results, B=4):
    full = np.empty((B, S, HID), dtype=np.float32)
    for c in range(N_CORES):
        b, hh = c // 2, c % 2
        full[b, :, hh * 384:(hh + 1) * 384] = results[c]["out"].T
    return full


_NC_CACHE = {}


def get_nc(ndev=N_CORES):
    if ndev not in _NC_CACHE:
        pairs = [[i, i + 1] for i in range(0, ndev, 2)]
        _NC_CACHE[ndev] = build_nc(ndev, pairs)
    return _NC_CACHE[ndev]


def kernel(**inputs):
    in_maps = prep_inputs(**inputs)
    nc = get_nc(N_CORES)
    res = bass_utils.run_bass_kernel_spmd(
        nc, in_maps, core_ids=list(range(N_CORES)))
    return assemble(res.results)
